# revision 26
# baseline (speedup 1.0000x reference)
"""Trainium2 Bass kernel for LocalDownsampleFlexAttn (24-head attention with
pooled-KV augmentation), head-parallel across 8 NeuronCores.

Sharding: each core owns 3 of the 24 heads. Per core:
  - QKV projections for its 3 heads (column-sliced Wq/Wk/Wv)
  - KV downsampling (4x4 spatial pooling of the 1024 image tokens -> 64)
  - attention over 1536+64 keys
  - partial output projection (row-sliced Wo); host sums the 8 partials + bo.

v2: all operands are pre-transposed/pre-cast to bf16 on the host (x arrives
as x^T so no PE transposes are needed), softmax denominators come from a DVE
reduce + one all-ones matmul partition-broadcast (no per-key-tile ones
matmuls, no DRAM roundtrip), the output is written fp16, and emission is
software-pipelined: scores of query-group g+1 are emitted before the PV of
group g, and the v-projection / output-projection matmuls fill the PE while
exp runs on the Scalar engine.
"""

import numpy as np
from contextlib import ExitStack

# ---- problem constants (hardcoded per harness contract) ----
S = 1536          # sequence length
DM = 3072         # model dim
NH = 24           # total heads
HD = 128          # head dim
NCORES = 8
HPC = NH // NCORES   # heads per core = 3
CW = HPC * HD        # per-core slice width = 384
TXT = 512
IMG = 1024        # image tokens (32x32)
F = 4             # pooling factor
PK = (IMG // (F * F))  # pooled keys = 64
KALL = S + PK     # 1600 keys
NKT = DM // 128   # 24 model-dim k-tiles
NTT = S // 128    # 12 token tiles
NIT = IMG // 128  # 8 image-token tiles
NKC = (KALL + 127) // 128   # 13 key tiles (last has 64)
NQG = S // 512    # 3 query groups of 512
ASCALE = float((1.0 / HD) ** 0.5)

_CACHE = {}


def _build_program():
    import concourse.bass as bass
    import concourse.bacc as bacc
    import concourse.tile as tile
    from concourse import mybir

    f32 = mybir.dt.float32
    bf16 = mybir.dt.bfloat16
    fp16 = mybir.dt.float16
    AF = mybir.ActivationFunctionType
    AX = mybir.AxisListType

    nc = bacc.Bacc(
        "TRN2",
        target_bir_lowering=False,
        debug=False,
        enable_asserts=False,
        num_devices=NCORES,
    )

    xt_d = nc.dram_tensor("xt", [DM, S], bf16, kind="ExternalInput").ap()
    wq_d = nc.dram_tensor("wq", [DM, CW], bf16, kind="ExternalInput").ap()
    wk_d = nc.dram_tensor("wk", [DM, CW], bf16, kind="ExternalInput").ap()
    wv_d = nc.dram_tensor("wv", [DM, CW], bf16, kind="ExternalInput").ap()
    bq_d = nc.dram_tensor("bq", [128, HPC], f32, kind="ExternalInput").ap()
    bk_d = nc.dram_tensor("bk", [128, HPC], f32, kind="ExternalInput").ap()
    bv_d = nc.dram_tensor("bv", [1, CW], bf16, kind="ExternalInput").ap()
    wo_d = nc.dram_tensor("wo", [128, HPC, DM], bf16, kind="ExternalInput").ap()
    pm_d = nc.dram_tensor("pm", [128, NIT * PK], bf16, kind="ExternalInput").ap()
    wfull_d = nc.dram_tensor("wfull", [IMG], bf16, kind="ExternalInput").ap()
    out_d = nc.dram_tensor("out", [S, DM], fp16, kind="ExternalOutput").ap()

    with tile.TileContext(nc) as tc, ExitStack() as ctx:
        persist = ctx.enter_context(tc.tile_pool(name="persist", bufs=1))

        ones128 = persist.tile([128, 128], bf16)
        nc.vector.memset(ones128, 1.0)
        ones_row = persist.tile([1, 128], bf16)
        nc.vector.memset(ones_row, 1.0)

        bq_sb = persist.tile([128, HPC], f32)
        bk_sb = persist.tile([128, HPC], f32)
        bvrow = persist.tile([1, CW], bf16)
        pm_bf = persist.tile([128, NIT, PK], bf16)
        wfull_sb = persist.tile([128, IMG], bf16)


        # persistent activations
        qT = persist.tile([128, HPC, S], bf16)          # q^T per head [d, tok]
        kT = persist.tile([128, HPC, NKC * 128], bf16)  # k_all^T per head [d, key]
        vA = persist.tile([128, HPC, NKC, HD], bf16)    # v_all per head [key, kt, d]
        attnT = persist.tile([128, HPC, S], bf16)       # attn^T [d(by head), tok]

        XCH = 4   # xT DMA granularity (k-tiles per DMA)
        WCH = 12  # W DMA granularity


        # -------- attention helpers (emission pieces, per (qg, head)) --------
        def sc_g(qg, h):
            qsl = slice(qg * 512, (qg + 1) * 512)
            pt = pCt.tile([128, NKC, 512], bf16, tag="pt", bufs=3,
                          name=f"pt{qg}h{h}")
            nc.vector.memset(pt[PK:, NKC - 1, :], 0.0)
            for c in range(NKC):
                cs = 128 if c < NKC - 1 else PK
                psc = pCsc.tile([128, 512], f32, tag="sc", bufs=2)
                nc.tensor.matmul(
                    psc[:cs, :],
                    kT[:, h, c * 128:c * 128 + cs],
                    qT[:, h, qsl],
                    start=True, stop=True,
                )
                nc.scalar.activation(
                    pt[:cs, c, :], psc[:cs, :], AF.Exp,
                    bias=0.0, scale=ASCALE,
                )
            return pt

        def tree_g(pool, pt):
            # softmax denominator partial sums: pairwise DVE tree over the
            # 13 key tiles (contiguous adds; the strided reduce was 10x slower)
            s = pool.tile([128, 6, 512], bf16, tag="tree", bufs=2)
            with nc.allow_low_precision(
                    reason="bf16 pairwise sums of positive probs; "
                           "0.4% on the softmax denominator is in budget"):
                nc.vector.tensor_add(s[:, 0:6, :], pt[:, 0:6, :], pt[:, 6:12, :])
                nc.vector.tensor_add(s[:, 0:3, :], s[:, 0:3, :], s[:, 3:6, :])
                nc.vector.tensor_add(s[:, 3, :], s[:, 0, :], s[:, 1, :])
                nc.vector.tensor_add(s[:, 4, :], s[:, 2, :], pt[:, 12, :])
                nc.vector.tensor_add(s[:, 5, :], s[:, 3, :], s[:, 4, :])
            return s[:, 5, :]

        def pv_g(qg, h, pt, sumt, rpool):
            qsl = slice(qg * 512, (qg + 1) * 512)
            bcp = pCbp.tile([128, 512], f32, tag="bc", bufs=2)
            nc.tensor.matmul(bcp, ones128, sumt, start=True, stop=True)
            rbc = rpool.tile([128, 512], f32, tag="rbc", bufs=2)
            nc.vector.reciprocal_approx_fast(out=rbc, in_=bcp)
            ppv = pCbp.tile([128, 512], f32, tag="pv", bufs=2)
            for c in range(NKC):
                cs = 128 if c < NKC - 1 else PK
                nc.tensor.matmul(
                    ppv,
                    vA[:cs, h, c, :],
                    pt[:cs, c, :],
                    start=(c == 0),
                    stop=(c == NKC - 1),
                )
            nc.vector.tensor_mul(attnT[:, h, qsl], ppv, rbc)

        def oproj_block(qg, wo_sb, p_out, p_op):
            for qt in range(qg * 4, (qg + 1) * 4):
                outsb = p_out.tile([128, DM], fp16, tag="outsb", bufs=2)
                for g in range(DM // 512):
                    pso = p_op.tile([128, 512], f32, tag="o", bufs=2)
                    for kt in range(HPC):
                        nc.tensor.matmul(
                            pso,
                            attnT[:, kt, qt * 128:(qt + 1) * 128],
                            wo_sb[:, kt, g * 512:(g + 1) * 512],
                            start=(kt == 0),
                            stop=(kt == HPC - 1),
                        )
                    if g % 2 == 0:
                        nc.vector.tensor_copy(
                            outsb[:, g * 512:(g + 1) * 512], pso)
                    else:
                        nc.scalar.copy(outsb[:, g * 512:(g + 1) * 512], pso)
                for hh in range(4):
                    nc.sync.dma_start(
                        out=out_d[qt * 128:(qt + 1) * 128,
                                  hh * 768:(hh + 1) * 768],
                        in_=outsb[:, hh * 768:(hh + 1) * 768])

        def load_w(dst, w_d, chunks=(12, 12), eng=None):
            eng = eng or nc.sync
            k0 = 0
            for n in chunks:
                eng.dma_start(
                    out=dst[:, k0:k0 + n, :],
                    in_=bass.AP(tensor=w_d.tensor, offset=k0 * 128 * CW,
                                ap=[[CW, 128], [128 * CW, n], [1, CW]]),
                )
                k0 += n

        # attention SBUF pools: must enclose the B pools on the stack
        # (first used between B1 and B2, live to the end of the program)
        pCt = ctx.enter_context(tc.tile_pool(name="pCt", bufs=1))
        pCsb = ctx.enter_context(tc.tile_pool(name="pCsb", bufs=1))

        with ExitStack() as bctx:
            pX = bctx.enter_context(tc.tile_pool(name="pX", bufs=1))
            pWA = bctx.enter_context(tc.tile_pool(name="pWA", bufs=1))
            xT = pX.tile([128, NKT, S], bf16)       # x^T [dm%128, dm//128, tok]
            wa_sb = pWA.tile([128, NKT, CW], bf16)  # wq, later wv

            k0 = 0
            for n in (1, 1, 2, 4, 4, 6, 6):
                nc.sync.dma_start(
                    out=wa_sb[:, k0:k0 + n, :],
                    in_=bass.AP(tensor=wq_d.tensor, offset=k0 * 128 * CW,
                                ap=[[CW, 128], [128 * CW, n], [1, CW]]),
                )
                nc.sync.dma_start(
                    out=xT[:, k0:k0 + n, :],
                    in_=bass.AP(tensor=xt_d.tensor, offset=k0 * 128 * S,
                                ap=[[S, 128], [128 * S, n], [1, S]]),
                )
                k0 += n

            # small persist loads go on the idle GpSimd DMA queue
            nc.gpsimd.dma_start(out=bq_sb, in_=bq_d)
            nc.gpsimd.dma_start(out=bk_sb, in_=bk_d)
            nc.gpsimd.dma_start(out=bvrow, in_=bv_d)
            nc.gpsimd.dma_start(
                out=pm_bf,
                in_=bass.AP(tensor=pm_d.tensor, offset=0,
                            ap=[[NIT * PK, 128], [PK, NIT], [1, PK]]),
            )
            nc.gpsimd.dma_start(
                out=wfull_sb,
                in_=bass.AP(tensor=wfull_d.tensor, offset=0,
                            ap=[[0, 128], [1, IMG]]),
            )

            # ---------------- Phase B1: q^T and k^T projections ----------------
            with tc.tile_pool(name="pWB", bufs=1) as pWB, \
                 tc.tile_pool(name="pBq", bufs=1, space="PSUM") as pBq:
                wb_sb = pWB.tile([128, NKT, CW], bf16)  # wk
                load_w(wb_sb, wk_d)
                # projection passes: 3 head-chains x 2 query-chunk columns
                # during the DMA-fed window consume xT at exactly the
                # delivery rate; the rest runs at full PE speed once
                # everything is resident
                def b1_pass(pname, w_sb, b_sb, dst, c_lo, c_n, pooled_k):
                    tiles = [pBq.tile([128, 1024], f32, tag="qkA", bufs=3,
                                      name=f"qk{pname}{h}")
                             for h in range(HPC)]
                    for kt in range(NKT):
                        for h in range(HPC):
                            for c in range(c_lo, c_lo + c_n):
                                nc.tensor.matmul(
                                    tiles[h][:, (c - c_lo) * 512:
                                             (c - c_lo + 1) * 512],
                                    w_sb[:, kt, h * 128:(h + 1) * 128],
                                    xT[:, kt, c * 512:(c + 1) * 512],
                                    start=(kt == 0),
                                    stop=(kt == NKT - 1),
                                )
                    for h in range(HPC):
                        nc.scalar.activation(
                            dst[:, h, c_lo * 512:(c_lo + c_n) * 512],
                            tiles[h][:, :c_n * 512], AF.Identity,
                            bias=b_sb[:, h:h + 1], scale=1.0,
                        )
                        if pooled_k:
                            do_pooled_k(h)

                def do_pooled_k(h):
                    if True:
                        if True:
                            # pooled k cols (kT[:, h, 1536:1600]) via DVE
                            tmpw = pWB.tile([128, IMG], f32, tag="tmpw", bufs=1)
                            for R in range(8):
                                nc.vector.tensor_mul(
                                    tmpw[:, R * 128:(R + 1) * 128].rearrange(
                                        "p (C i j) -> p C i j", C=8, i=4),
                                    kT[:, h,
                                       TXT + R * 128:TXT + (R + 1) * 128].rearrange(
                                        "p (i C j) -> p C i j", i=4, C=8),
                                    wfull_sb[:, R * 128:(R + 1) * 128].rearrange(
                                        "p (i C j) -> p C i j", i=4, C=8),
                                )
                            pooled = pWB.tile([128, PK], f32, tag="pooled", bufs=2)
                            nc.vector.reduce_sum(
                                pooled,
                                tmpw.rearrange("p (rc ij) -> p rc ij", ij=F * F),
                                axis=AX.X,
                            )
                            nc.vector.tensor_copy(kT[:, h, S:S + PK], pooled)

                b1_pass("qa", wa_sb, bq_sb, qT, 0, 2, False)
                b1_pass("qb", wa_sb, bq_sb, qT, 2, 1, False)
                b1_pass("ka", wb_sb, bk_sb, kT, 0, 2, False)
                b1_pass("kb", wb_sb, bk_sb, kT, 2, 1, True)

            # scores PSUM pool: opened after pBq popped, lives to the end
            pCsc = ctx.enter_context(
                tc.tile_pool(name="pCsc", bufs=1, space="PSUM"))

            # ------------- Phase C0: scores for query group 0 -------------
            t00 = sc_g(0, 0)
            t01 = sc_g(0, 1)
            t02 = sc_g(0, 2)

            # bc/pv PSUM pool: opened before B2 so the C/D boundary has no
            # pool-transition barrier (stack: pCsc, pCbp, pBv(pop), pDp)
            pCbp = ctx.enter_context(
                tc.tile_pool(name="pCbp", bufs=1, space="PSUM"))

            # ------------- Phase B2: v projection (+ pooled v) -------------
            with tc.tile_pool(name="pBv", bufs=1, space="PSUM") as pBv:
                load_w(wa_sb, wv_d)  # reuse wq tiles (q chains are done)
                for tt in range(NTT):
                    psv = pBv.tile([128, CW], f32, tag="v", bufs=2)
                    nc.tensor.matmul(psv, ones_row, bvrow, start=True, stop=False)
                    for kt in range(NKT):
                        nc.tensor.matmul(
                            psv,
                            xT[:, kt, tt * 128:(tt + 1) * 128],
                            wa_sb[:, kt, :],
                            start=False,
                            stop=(kt == NKT - 1),
                        )
                    nc.vector.tensor_copy(
                        vA[:, :, tt, :],
                        psv.rearrange("p (h d) -> p h d", h=HPC))
                s00 = tree_g(pCsb, t00)
                s01 = tree_g(pCsb, t01)
                for h in range(HPC):
                    psp = pBv.tile([128, CW], f32, tag="v", bufs=2)
                    for it in range(NIT):
                        nc.tensor.matmul(
                            psp[:PK, :HD],
                            pm_bf[:, it, :],
                            vA[:, h, (TXT // 128) + it, :],
                            start=(it == 0),
                            stop=(it == NIT - 1),
                        )
                    nc.vector.tensor_copy(vA[:PK, h, NKC - 1, :], psp[:PK, :HD])

        # ------------- Phases C/D: pipelined attention + out-proj -------------
        with tc.tile_pool(name="pD", bufs=1) as pD, \
             tc.tile_pool(name="pDpsum", bufs=1, space="PSUM") as pDp:
            wo_sb = pD.tile([128, HPC, DM], bf16)
            nc.sync.dma_start(
                out=wo_sb,
                in_=bass.AP(tensor=wo_d.tensor, offset=0,
                            ap=[[HPC * DM, 128], [DM, HPC], [1, DM]]),
            )
            pv_g(0, 0, t00, s00, pD)
            t10 = sc_g(1, 0)
            s02 = tree_g(pCsb, t02)
            pv_g(0, 1, t01, s01, pD)
            t11 = sc_g(1, 1)
            pv_g(0, 2, t02, s02, pD)
            t12 = sc_g(1, 2)
            s10 = tree_g(pCsb, t10)
            oproj_block(0, wo_sb, pD, pDp)
            pv_g(1, 0, t10, s10, pD)
            t20 = sc_g(2, 0)
            s11 = tree_g(pCsb, t11)
            pv_g(1, 1, t11, s11, pD)
            t21 = sc_g(2, 1)
            s12 = tree_g(pCsb, t12)
            pv_g(1, 2, t12, s12, pD)
            t22 = sc_g(2, 2)
            s20 = tree_g(pCsb, t20)
            oproj_block(1, wo_sb, pD, pDp)
            pv_g(2, 0, t20, s20, pD)
            s21 = tree_g(pCsb, t21)
            pv_g(2, 1, t21, s21, pD)
            s22 = tree_g(pCsb, t22)
            pv_g(2, 2, t22, s22, pD)
            oproj_block(2, wo_sb, pD, pDp)

    nc.compile()
    return nc


def _get_program():
    if "nc" not in _CACHE:
        _CACHE["nc"] = _build_program()
    return _CACHE["nc"]


def _prep_in_maps(hidden_states, Wq, bq, Wk, bk, Wv, bv, Wo, spatial_weight):
    import ml_dtypes
    bf16 = ml_dtypes.bfloat16

    x = np.asarray(hidden_states, dtype=np.float32).reshape(S, DM)
    xt = np.ascontiguousarray(x.T.astype(bf16))
    Wq = np.asarray(Wq, dtype=np.float32)
    Wk = np.asarray(Wk, dtype=np.float32)
    Wv = np.asarray(Wv, dtype=np.float32)
    Wo = np.asarray(Wo, dtype=np.float32)
    bq = np.asarray(bq, dtype=np.float32)
    bk = np.asarray(bk, dtype=np.float32)
    bv = np.asarray(bv, dtype=np.float32)

    w = np.asarray(spatial_weight, dtype=np.float32).reshape(F, F)  # [i, j]
    # wfull[t] for t = 128R + 32i + 4C + j  -> broadcast w over (R, C)
    wfull = np.ascontiguousarray(
        np.broadcast_to(w[None, :, None, :], (8, F, 8, F)).reshape(IMG)
        .astype(bf16)
    )
    # pmat[t, R*8+C] = w[i, j] for t in block (R, C)
    pmat = np.zeros((8, F, 8, F, 8, 8), dtype=np.float32)
    for R in range(8):
        for C in range(8):
            pmat[R, :, C, :, R, C] = w
    pmat = pmat.reshape(IMG, PK)
    # pack [it*128+p, j] -> [p, it*PK+j]
    pm_packed = np.ascontiguousarray(
        pmat.reshape(NIT, 128, PK).transpose(1, 0, 2).reshape(128, NIT * PK)
        .astype(bf16))

    in_maps = []
    for c in range(NCORES):
        sl = slice(c * CW, (c + 1) * CW)
        wo_sl = np.ascontiguousarray(  # [128, HPC, DM]
            Wo[sl, :].reshape(HPC, 128, DM).transpose(1, 0, 2).astype(bf16))
        in_maps.append({
            "xt": xt,
            "wq": np.ascontiguousarray(Wq[:, sl].astype(bf16)),
            "wk": np.ascontiguousarray(Wk[:, sl].astype(bf16)),
            "wv": np.ascontiguousarray(Wv[:, sl].astype(bf16)),
            "bq": np.ascontiguousarray(bq[sl].reshape(HPC, 128).T),
            "bk": np.ascontiguousarray(bk[sl].reshape(HPC, 128).T),
            "bv": np.ascontiguousarray(bv[sl].astype(bf16).reshape(1, CW)),
            "wo": wo_sl,
            "pm": pm_packed,
            "wfull": wfull,
        })
    return in_maps


def _run(inputs, trace=False, trace_kwargs=None):
    from concourse import bass_utils

    nc = _get_program()
    in_maps = _prep_in_maps(
        inputs["hidden_states"], inputs["Wq"], inputs["bq"], inputs["Wk"],
        inputs["bk"], inputs["Wv"], inputs["bv"], inputs["Wo"],
        inputs["spatial_weight"],
    )
    res = bass_utils.run_bass_kernel_spmd(
        nc, in_maps, list(range(NCORES)), trace=trace,
        **(trace_kwargs or {}),
    )
    partial = np.zeros((S, DM), dtype=np.float32)
    for r in res.results:
        partial += r["out"].astype(np.float32)
    out = partial + np.asarray(inputs["bo"], dtype=np.float32)[None, :]
    return out.reshape(1, S, DM).astype(np.float32), res


def kernel(**inputs):
    h = int(inputs.get("height", 32))
    w = int(inputs.get("width", 32))
    assert h == 32 and w == 32, (h, w)
    out, _ = _run(inputs, trace=False)
    return out


# revision 29
# speedup vs baseline: 1.0158x; 1.0158x over previous
"""Trainium2 Bass kernel for LocalDownsampleFlexAttn (24-head attention with
pooled-KV augmentation), head-parallel across 8 NeuronCores.

Sharding: each core owns 3 of the 24 heads. Per core:
  - QKV projections for its 3 heads (column-sliced Wq/Wk/Wv)
  - KV downsampling (4x4 spatial pooling of the 1024 image tokens -> 64)
  - attention over 1536+64 keys
  - partial output projection (row-sliced Wo); host sums the 8 partials + bo.

All operands are pre-transposed/pre-cast to bf16 on the host (x arrives as
x^T so no PE transposes or casts are needed on device). Softmax denominators
come from a contiguous DVE pairwise-add tree over the 13 key tiles plus one
all-ones matmul that sums over partitions and broadcasts in a single pass;
1/denom uses the fast approximate reciprocal. The output is written fp16.
Emission is software-pipelined: the q/k projection chains run three heads
abreast at the DMA delivery rate of the streamed x^T, scores of query group
g+1 are emitted before the PV of group g, and the v-projection /
output-projection matmuls keep the PE busy while exp runs on Scalar.
"""

import numpy as np
from contextlib import ExitStack

# ---- problem constants (hardcoded per harness contract) ----
S = 1536          # sequence length
DM = 3072         # model dim
NH = 24           # total heads
HD = 128          # head dim
NCORES = 8
HPC = NH // NCORES   # heads per core = 3
CW = HPC * HD        # per-core slice width = 384
TXT = 512
IMG = 1024        # image tokens (32x32)
F = 4             # pooling factor
PK = (IMG // (F * F))  # pooled keys = 64
KALL = S + PK     # 1600 keys
NKT = DM // 128   # 24 model-dim k-tiles
NTT = S // 128    # 12 token tiles
NIT = IMG // 128  # 8 image-token tiles
NKC = (KALL + 127) // 128   # 13 key tiles (last has 64)
NQG = S // 512    # 3 query groups of 512
ASCALE = float((1.0 / HD) ** 0.5)

_CACHE = {}


def _build_program():
    import concourse.bass as bass
    import concourse.bacc as bacc
    import concourse.tile as tile
    from concourse import mybir

    f32 = mybir.dt.float32
    bf16 = mybir.dt.bfloat16
    fp16 = mybir.dt.float16
    AF = mybir.ActivationFunctionType
    AX = mybir.AxisListType

    nc = bacc.Bacc(
        "TRN2",
        target_bir_lowering=False,
        debug=False,
        enable_asserts=False,
        num_devices=NCORES,
    )

    xt_d = nc.dram_tensor("xt", [DM, S], bf16, kind="ExternalInput").ap()
    wq_d = nc.dram_tensor("wq", [DM, CW], bf16, kind="ExternalInput").ap()
    wk_d = nc.dram_tensor("wk", [DM, CW], bf16, kind="ExternalInput").ap()
    wv_d = nc.dram_tensor("wv", [DM, CW], bf16, kind="ExternalInput").ap()
    bq_d = nc.dram_tensor("bq", [128, HPC], f32, kind="ExternalInput").ap()
    bk_d = nc.dram_tensor("bk", [128, HPC], f32, kind="ExternalInput").ap()
    bv_d = nc.dram_tensor("bv", [1, CW], bf16, kind="ExternalInput").ap()
    wo_d = nc.dram_tensor("wo", [128, HPC, DM], bf16, kind="ExternalInput").ap()
    pm_d = nc.dram_tensor("pm", [128, NIT * PK], bf16, kind="ExternalInput").ap()
    wfull_d = nc.dram_tensor("wfull", [IMG], bf16, kind="ExternalInput").ap()
    out_d = nc.dram_tensor("out", [S, DM], fp16, kind="ExternalOutput").ap()

    with tile.TileContext(nc) as tc, ExitStack() as ctx:
        persist = ctx.enter_context(tc.tile_pool(name="persist", bufs=1))

        ones128 = persist.tile([128, 128], bf16)
        nc.vector.memset(ones128, 1.0)
        ones_row = persist.tile([1, 128], bf16)
        nc.vector.memset(ones_row, 1.0)

        bq_sb = persist.tile([128, HPC], f32)
        bk_sb = persist.tile([128, HPC], f32)
        bvrow = persist.tile([1, CW], bf16)
        pm_bf = persist.tile([128, NIT, PK], bf16)
        wfull_sb = persist.tile([128, IMG], bf16)


        # persistent activations
        qT = persist.tile([128, HPC, S], bf16)          # q^T per head [d, tok]
        kT = persist.tile([128, HPC, NKC * 128], bf16)  # k_all^T per head [d, key]
        vA = persist.tile([128, HPC, NKC, HD], bf16)    # v_all per head [key, kt, d]
        attnT = persist.tile([128, HPC, S], bf16)       # attn^T [d(by head), tok]

        XCH = 4   # xT DMA granularity (k-tiles per DMA)
        WCH = 12  # W DMA granularity


        # -------- attention helpers (emission pieces, per (qg, head)) --------
        def sc_g(qg, h):
            qsl = slice(qg * 512, (qg + 1) * 512)
            pt = pCt.tile([128, NKC, 512], bf16, tag="pt", bufs=3,
                          name=f"pt{qg}h{h}")
            nc.vector.memset(pt[PK:, NKC - 1, :], 0.0)
            for c in range(NKC):
                cs = 128 if c < NKC - 1 else PK
                psc = pCsc.tile([128, 512], f32, tag="sc", bufs=2)
                nc.tensor.matmul(
                    psc[:cs, :],
                    kT[:, h, c * 128:c * 128 + cs],
                    qT[:, h, qsl],
                    start=True, stop=True,
                )
                nc.scalar.activation(
                    pt[:cs, c, :], psc[:cs, :], AF.Exp,
                    bias=0.0, scale=ASCALE,
                )
            return pt

        def tree_g(pool, pt):
            # softmax denominator partial sums: pairwise DVE tree over the
            # 13 key tiles (contiguous adds; the strided reduce was 10x slower)
            s = pool.tile([128, 6, 512], bf16, tag="tree", bufs=2)
            with nc.allow_low_precision(
                    reason="bf16 pairwise sums of positive probs; "
                           "0.4% on the softmax denominator is in budget"):
                nc.vector.tensor_add(s[:, 0:6, :], pt[:, 0:6, :], pt[:, 6:12, :])
                nc.vector.tensor_add(s[:, 0:3, :], s[:, 0:3, :], s[:, 3:6, :])
                nc.vector.tensor_add(s[:, 3, :], s[:, 0, :], s[:, 1, :])
                nc.vector.tensor_add(s[:, 4, :], s[:, 2, :], pt[:, 12, :])
                nc.vector.tensor_add(s[:, 5, :], s[:, 3, :], s[:, 4, :])
            return s[:, 5, :]

        def pv_g(qg, h, pt, sumt, rpool):
            qsl = slice(qg * 512, (qg + 1) * 512)
            bcp = pCbp.tile([128, 512], f32, tag="bc", bufs=2)
            nc.tensor.matmul(bcp, ones128, sumt, start=True, stop=True)
            rbc = rpool.tile([128, 512], f32, tag="rbc", bufs=2)
            nc.vector.reciprocal_approx_fast(out=rbc, in_=bcp)
            ppv = pCbp.tile([128, 512], f32, tag="pv", bufs=2)
            for c in range(NKC):
                cs = 128 if c < NKC - 1 else PK
                nc.tensor.matmul(
                    ppv,
                    vA[:cs, h, c, :],
                    pt[:cs, c, :],
                    start=(c == 0),
                    stop=(c == NKC - 1),
                )
            nc.vector.tensor_mul(attnT[:, h, qsl], ppv, rbc)

        def oproj_block(qg, wo_sb, p_out, p_op):
            for qt in range(qg * 4, (qg + 1) * 4):
                outsb = p_out.tile([128, DM], fp16, tag="outsb", bufs=2)
                for g in range(DM // 512):
                    pso = p_op.tile([128, 512], f32, tag="o", bufs=2)
                    for kt in range(HPC):
                        nc.tensor.matmul(
                            pso,
                            attnT[:, kt, qt * 128:(qt + 1) * 128],
                            wo_sb[:, kt, g * 512:(g + 1) * 512],
                            start=(kt == 0),
                            stop=(kt == HPC - 1),
                        )
                    if g % 2 == 0:
                        nc.vector.tensor_copy(
                            outsb[:, g * 512:(g + 1) * 512], pso)
                    else:
                        nc.scalar.copy(outsb[:, g * 512:(g + 1) * 512], pso)
                for hh in range(4):
                    nc.sync.dma_start(
                        out=out_d[qt * 128:(qt + 1) * 128,
                                  hh * 768:(hh + 1) * 768],
                        in_=outsb[:, hh * 768:(hh + 1) * 768])

        def load_w(dst, w_d, chunks=(12, 12), eng=None):
            eng = eng or nc.sync
            k0 = 0
            for n in chunks:
                eng.dma_start(
                    out=dst[:, k0:k0 + n, :],
                    in_=bass.AP(tensor=w_d.tensor, offset=k0 * 128 * CW,
                                ap=[[CW, 128], [128 * CW, n], [1, CW]]),
                )
                k0 += n

        # attention SBUF pools: must enclose the B pools on the stack
        # (first used between B1 and B2, live to the end of the program)
        pCt = ctx.enter_context(tc.tile_pool(name="pCt", bufs=1))
        pCsb = ctx.enter_context(tc.tile_pool(name="pCsb", bufs=1))

        with ExitStack() as bctx:
            pX = bctx.enter_context(tc.tile_pool(name="pX", bufs=1))
            pWA = bctx.enter_context(tc.tile_pool(name="pWA", bufs=1))
            xT = pX.tile([128, NKT, S], bf16)       # x^T [dm%128, dm//128, tok]
            wa_sb = pWA.tile([128, NKT, CW], bf16)  # wq, later wv

            k0 = 0
            for n in (2, 4, 6, 6, 6):
                nc.sync.dma_start(
                    out=wa_sb[:, k0:k0 + n, :],
                    in_=bass.AP(tensor=wq_d.tensor, offset=k0 * 128 * CW,
                                ap=[[CW, 128], [128 * CW, n], [1, CW]]),
                )
                nc.sync.dma_start(
                    out=xT[:, k0:k0 + n, :],
                    in_=bass.AP(tensor=xt_d.tensor, offset=k0 * 128 * S,
                                ap=[[S, 128], [128 * S, n], [1, S]]),
                )
                k0 += n

            # small persist loads go on the idle GpSimd DMA queue
            nc.gpsimd.dma_start(out=bq_sb, in_=bq_d)
            nc.gpsimd.dma_start(out=bk_sb, in_=bk_d)
            nc.gpsimd.dma_start(out=bvrow, in_=bv_d)
            nc.gpsimd.dma_start(
                out=pm_bf,
                in_=bass.AP(tensor=pm_d.tensor, offset=0,
                            ap=[[NIT * PK, 128], [PK, NIT], [1, PK]]),
            )
            nc.gpsimd.dma_start(
                out=wfull_sb,
                in_=bass.AP(tensor=wfull_d.tensor, offset=0,
                            ap=[[0, 128], [1, IMG]]),
            )

            # ---------------- Phase B1: q^T and k^T projections ----------------
            with tc.tile_pool(name="pWB", bufs=1) as pWB, \
                 tc.tile_pool(name="pBq", bufs=1, space="PSUM") as pBq:
                wb_sb = pWB.tile([128, NKT, CW], bf16)  # wk
                load_w(wb_sb, wk_d)
                # projection passes: 3 head-chains x 2 query-chunk columns
                # during the DMA-fed window consume xT at exactly the
                # delivery rate; the rest runs at full PE speed once
                # everything is resident
                def b1_pass(pname, w_sb, b_sb, dst, c_lo, c_n, pooled_k):
                    tiles = [pBq.tile([128, 1024], f32, tag="qkA", bufs=3,
                                      name=f"qk{pname}{h}")
                             for h in range(HPC)]
                    for kt in range(NKT):
                        for h in range(HPC):
                            for c in range(c_lo, c_lo + c_n):
                                nc.tensor.matmul(
                                    tiles[h][:, (c - c_lo) * 512:
                                             (c - c_lo + 1) * 512],
                                    w_sb[:, kt, h * 128:(h + 1) * 128],
                                    xT[:, kt, c * 512:(c + 1) * 512],
                                    start=(kt == 0),
                                    stop=(kt == NKT - 1),
                                )
                    for h in range(HPC):
                        nc.scalar.activation(
                            dst[:, h, c_lo * 512:(c_lo + c_n) * 512],
                            tiles[h][:, :c_n * 512], AF.Identity,
                            bias=b_sb[:, h:h + 1], scale=1.0,
                        )
                        if pooled_k:
                            do_pooled_k(h)

                def do_pooled_k(h):
                    if True:
                        if True:
                            # pooled k cols (kT[:, h, 1536:1600]) via DVE
                            tmpw = pWB.tile([128, IMG], f32, tag="tmpw", bufs=1)
                            for R in range(8):
                                nc.vector.tensor_mul(
                                    tmpw[:, R * 128:(R + 1) * 128].rearrange(
                                        "p (C i j) -> p C i j", C=8, i=4),
                                    kT[:, h,
                                       TXT + R * 128:TXT + (R + 1) * 128].rearrange(
                                        "p (i C j) -> p C i j", i=4, C=8),
                                    wfull_sb[:, R * 128:(R + 1) * 128].rearrange(
                                        "p (i C j) -> p C i j", i=4, C=8),
                                )
                            pooled = pWB.tile([128, PK], f32, tag="pooled", bufs=2)
                            nc.vector.reduce_sum(
                                pooled,
                                tmpw.rearrange("p (rc ij) -> p rc ij", ij=F * F),
                                axis=AX.X,
                            )
                            nc.vector.tensor_copy(kT[:, h, S:S + PK], pooled)

                b1_pass("qa", wa_sb, bq_sb, qT, 0, 2, False)
                b1_pass("qb", wa_sb, bq_sb, qT, 2, 1, False)
                b1_pass("ka", wb_sb, bk_sb, kT, 0, 2, False)
                b1_pass("kb", wb_sb, bk_sb, kT, 2, 1, True)

            # scores PSUM pool: opened after pBq popped, lives to the end
            pCsc = ctx.enter_context(
                tc.tile_pool(name="pCsc", bufs=1, space="PSUM"))

            # ------------- Phase C0: scores for query group 0 -------------
            t00 = sc_g(0, 0)
            t01 = sc_g(0, 1)
            t02 = sc_g(0, 2)

            # bc/pv PSUM pool: opened before B2 so the C/D boundary has no
            # pool-transition barrier (stack: pCsc, pCbp, pBv(pop), pDp)
            pCbp = ctx.enter_context(
                tc.tile_pool(name="pCbp", bufs=1, space="PSUM"))

            # ------------- Phase B2: v projection (+ pooled v) -------------
            with tc.tile_pool(name="pBv", bufs=1, space="PSUM") as pBv:
                load_w(wa_sb, wv_d)  # reuse wq tiles (q chains are done)
                for tt in range(NTT):
                    psv = pBv.tile([128, CW], f32, tag="v", bufs=2)
                    nc.tensor.matmul(psv, ones_row, bvrow, start=True, stop=False)
                    for kt in range(NKT):
                        nc.tensor.matmul(
                            psv,
                            xT[:, kt, tt * 128:(tt + 1) * 128],
                            wa_sb[:, kt, :],
                            start=False,
                            stop=(kt == NKT - 1),
                        )
                    nc.vector.tensor_copy(
                        vA[:, :, tt, :],
                        psv.rearrange("p (h d) -> p h d", h=HPC))
                s00 = tree_g(pCsb, t00)
                s01 = tree_g(pCsb, t01)
                for h in range(HPC):
                    psp = pBv.tile([128, CW], f32, tag="v", bufs=2)
                    for it in range(NIT):
                        nc.tensor.matmul(
                            psp[:PK, :HD],
                            pm_bf[:, it, :],
                            vA[:, h, (TXT // 128) + it, :],
                            start=(it == 0),
                            stop=(it == NIT - 1),
                        )
                    nc.vector.tensor_copy(vA[:PK, h, NKC - 1, :], psp[:PK, :HD])

        # ------------- Phases C/D: pipelined attention + out-proj -------------
        with tc.tile_pool(name="pD", bufs=1) as pD, \
             tc.tile_pool(name="pDpsum", bufs=1, space="PSUM") as pDp:
            wo_sb = pD.tile([128, HPC, DM], bf16)
            nc.sync.dma_start(
                out=wo_sb,
                in_=bass.AP(tensor=wo_d.tensor, offset=0,
                            ap=[[HPC * DM, 128], [DM, HPC], [1, DM]]),
            )
            pv_g(0, 0, t00, s00, pD)
            t10 = sc_g(1, 0)
            s02 = tree_g(pCsb, t02)
            pv_g(0, 1, t01, s01, pD)
            t11 = sc_g(1, 1)
            pv_g(0, 2, t02, s02, pD)
            t12 = sc_g(1, 2)
            s10 = tree_g(pCsb, t10)
            oproj_block(0, wo_sb, pD, pDp)
            pv_g(1, 0, t10, s10, pD)
            t20 = sc_g(2, 0)
            s11 = tree_g(pCsb, t11)
            pv_g(1, 1, t11, s11, pD)
            t21 = sc_g(2, 1)
            s12 = tree_g(pCsb, t12)
            pv_g(1, 2, t12, s12, pD)
            t22 = sc_g(2, 2)
            s20 = tree_g(pCsb, t20)
            oproj_block(1, wo_sb, pD, pDp)
            pv_g(2, 0, t20, s20, pD)
            s21 = tree_g(pCsb, t21)
            pv_g(2, 1, t21, s21, pD)
            s22 = tree_g(pCsb, t22)
            pv_g(2, 2, t22, s22, pD)
            oproj_block(2, wo_sb, pD, pDp)

    nc.compile()
    return nc


def _get_program():
    if "nc" not in _CACHE:
        _CACHE["nc"] = _build_program()
    return _CACHE["nc"]


def _prep_in_maps(hidden_states, Wq, bq, Wk, bk, Wv, bv, Wo, spatial_weight):
    import ml_dtypes
    bf16 = ml_dtypes.bfloat16

    x = np.asarray(hidden_states, dtype=np.float32).reshape(S, DM)
    xt = np.ascontiguousarray(x.T.astype(bf16))
    Wq = np.asarray(Wq, dtype=np.float32)
    Wk = np.asarray(Wk, dtype=np.float32)
    Wv = np.asarray(Wv, dtype=np.float32)
    Wo = np.asarray(Wo, dtype=np.float32)
    bq = np.asarray(bq, dtype=np.float32)
    bk = np.asarray(bk, dtype=np.float32)
    bv = np.asarray(bv, dtype=np.float32)

    w = np.asarray(spatial_weight, dtype=np.float32).reshape(F, F)  # [i, j]
    # wfull[t] for t = 128R + 32i + 4C + j  -> broadcast w over (R, C)
    wfull = np.ascontiguousarray(
        np.broadcast_to(w[None, :, None, :], (8, F, 8, F)).reshape(IMG)
        .astype(bf16)
    )
    # pmat[t, R*8+C] = w[i, j] for t in block (R, C)
    pmat = np.zeros((8, F, 8, F, 8, 8), dtype=np.float32)
    for R in range(8):
        for C in range(8):
            pmat[R, :, C, :, R, C] = w
    pmat = pmat.reshape(IMG, PK)
    # pack [it*128+p, j] -> [p, it*PK+j]
    pm_packed = np.ascontiguousarray(
        pmat.reshape(NIT, 128, PK).transpose(1, 0, 2).reshape(128, NIT * PK)
        .astype(bf16))

    in_maps = []
    for c in range(NCORES):
        sl = slice(c * CW, (c + 1) * CW)
        wo_sl = np.ascontiguousarray(  # [128, HPC, DM]
            Wo[sl, :].reshape(HPC, 128, DM).transpose(1, 0, 2).astype(bf16))
        in_maps.append({
            "xt": xt,
            "wq": np.ascontiguousarray(Wq[:, sl].astype(bf16)),
            "wk": np.ascontiguousarray(Wk[:, sl].astype(bf16)),
            "wv": np.ascontiguousarray(Wv[:, sl].astype(bf16)),
            "bq": np.ascontiguousarray(bq[sl].reshape(HPC, 128).T),
            "bk": np.ascontiguousarray(bk[sl].reshape(HPC, 128).T),
            "bv": np.ascontiguousarray(bv[sl].astype(bf16).reshape(1, CW)),
            "wo": wo_sl,
            "pm": pm_packed,
            "wfull": wfull,
        })
    return in_maps


def _run(inputs, trace=False, trace_kwargs=None):
    from concourse import bass_utils

    nc = _get_program()
    in_maps = _prep_in_maps(
        inputs["hidden_states"], inputs["Wq"], inputs["bq"], inputs["Wk"],
        inputs["bk"], inputs["Wv"], inputs["bv"], inputs["Wo"],
        inputs["spatial_weight"],
    )
    res = bass_utils.run_bass_kernel_spmd(
        nc, in_maps, list(range(NCORES)), trace=trace,
        **(trace_kwargs or {}),
    )
    partial = np.zeros((S, DM), dtype=np.float32)
    for r in res.results:
        partial += r["out"].astype(np.float32)
    out = partial + np.asarray(inputs["bo"], dtype=np.float32)[None, :]
    return out.reshape(1, S, DM).astype(np.float32), res


def kernel(**inputs):
    h = int(inputs.get("height", 32))
    w = int(inputs.get("width", 32))
    assert h == 32 and w == 32, (h, w)
    out, _ = _run(inputs, trace=False)
    return out
